# revision 13
# baseline (speedup 1.0000x reference)
"""CrossCoderDecoder forward on 8 trn2 NeuronCores.

x[b,l,d] = sum_f f[b,f] * weight[l,f,d] + bias[l,d]
B=32, L=2, F=65536, D=768, fp32.

Sharding: the F (dict) axis is split 8 ways (8192 features per core).
Each core computes its partial [L, 2*B, D] sums; the host sums the 8
partials (and the hi/lo half-pair, see below) and adds the bias (the
"all-reduce" of the sharding hint, done host-side since the output is
tiny).

Precision/perf scheme: fp32 matmuls on trn2 lower to 2 half-rate PE
passes (4 cycles/row), which makes the PE the bottleneck (~170us vs
~142us of DMA). Instead each fp32 operand is split hi/lo into two
bf16 tensors (x = xh + xl, xl = bf16(x - f32(xh))), the PE stationary
operand packs fh and fl side by side ([128, 64] lhsT), and the weight
stream runs twice (wh then wl) accumulating into a [64, N] PSUM tile:

    psum[0:32]  += fh.wh + fh.wl
    psum[32:64] += fl.wh + fl.wl

i.e. all four cross terms in 2 bf16 streaming passes (2 cycles/row on
the PE vs fp32's 4), same total DMA bytes as fp32 (2+2 vs 4 B/elem).
The host adds psum[0:32] + psum[32:64] during the partial reduction.
End-to-end output error is ~4e-6 relative (fp32-grade).

Weight DMA layout: per (l, chunk of CHUNK_ROWS k-rows) one dma_start
moves a contiguous block into SBUF [128, KO, 2, D] bf16 such that
each partition reads one contiguous line. The host pre-packs the
weights into exactly that image (hi/lo interleaved per k-row), and
pre-permutes f into fhl[p, j, 64] with the matching k order, so the
contraction stays consistent.
"""

import numpy as np
import ml_dtypes

import concourse.bass as bass
import concourse.tile as tile
from concourse import bacc, mybir
from concourse import bass_utils

B, L, F, D = 32, 2, 65536, 768
NCORES = 8
FS = F // NCORES          # 8192 features per core
P = 128
CHUNK_ROWS = 256          # k-rows per weight DMA
CH = FS // CHUNK_ROWS     # chunks per l
KO = CHUNK_ROWS // P      # k-subtiles per chunk
W_BUFS = 20               # weight tile double-buffering depth
W_SINGLE_RING = False     # True: all w DMAs on the SP ring (slower)
NSPLITS = ((0, 512), (512, 768))  # PSUM-bank splits of D

_F32 = mybir.dt.float32
_BF16 = mybir.dt.bfloat16
_BF16_NP = ml_dtypes.bfloat16

_cache = {}


def set_tiling(chunk_rows: int, w_bufs: int | None = None):
    """Adjust chunking (for tuning sweeps); drops the cached program."""
    global CHUNK_ROWS, CH, KO, W_BUFS
    CHUNK_ROWS = chunk_rows
    CH = FS // CHUNK_ROWS
    KO = CHUNK_ROWS // P
    if w_bufs is not None:
        W_BUFS = w_bufs
    _cache.clear()


def _build():
    """Build + schedule the (per-core identical) Bass program once."""
    nc = bacc.Bacc("TRN2", target_bir_lowering=False, debug=False)

    fhl = nc.dram_tensor("fhl", [P, CH * KO, 2 * B], _BF16, kind="ExternalInput").ap()
    w = nc.dram_tensor("w", [L, CH, P, KO, 2, D], _BF16, kind="ExternalInput").ap()
    out = nc.dram_tensor("out", [L, 2 * B, D], _F32, kind="ExternalOutput").ap()

    with tile.TileContext(nc) as tc:
        with (
            tc.tile_pool(name="fpool", bufs=1) as fpool,
            tc.tile_pool(name="wpool", bufs=W_BUFS) as wpool,
            tc.tile_pool(name="opool", bufs=2) as opool,
            tc.tile_pool(name="psum", bufs=1, space="PSUM") as psum,
        ):
            # fhl rides the ACT HWDGE ring so it overlaps the first w
            # chunks (the SP ring is FIFO per issuing engine).
            f_sb = fpool.tile([P, CH * KO, 2 * B], _BF16)
            nc.scalar.dma_start(f_sb[:], fhl[:])

            # Both l-groups' PSUM accumulators stay open for the whole
            # kernel; chunks interleave l so the DMA stream never hits a
            # drain point until the very end.
            ps = [
                [
                    psum.tile([2 * B, n1 - n0], _F32, name=f"ps_{l}_{i}")
                    for i, (n0, n1) in enumerate(NSPLITS)
                ]
                for l in range(L)
            ]
            for ch in range(CH):
                for l in range(L):
                    wt = wpool.tile([P, KO, 2, D], _BF16)
                    dma_eng = (
                        nc.sync
                        if (W_SINGLE_RING or (ch * L + l) % 2 == 0)
                        else nc.scalar
                    )
                    dma_eng.dma_start(wt[:], w[l, ch])
                    for o in range(KO):
                        j = ch * KO + o
                        first = j == 0
                        last = j == CH * KO - 1
                        for wi in (0, 1):  # wh pass, wl pass
                            for i, (n0, n1) in enumerate(NSPLITS):
                                nc.tensor.matmul(
                                    ps[l][i][:],
                                    f_sb[:, j, :],
                                    wt[:, o, wi, n0:n1],
                                    start=first and wi == 0,
                                    stop=last and wi == 1,
                                )
            for l in range(L):
                out_sb = opool.tile([2 * B, D], _F32)
                for i, (n0, n1) in enumerate(NSPLITS):
                    nc.vector.tensor_copy(out=out_sb[:, n0:n1], in_=ps[l][i][:])
                nc.scalar.dma_start(out[l], out_sb[:])

    nc.compile()
    return nc


def _split_hl(x: np.ndarray):
    """fp32 -> (hi, lo) bf16 pair with x ~= hi + lo."""
    hi = x.astype(_BF16_NP)
    lo = (x - hi.astype(np.float32)).astype(_BF16_NP)
    return hi, lo


def _prep_f(f_core: np.ndarray) -> np.ndarray:
    """f_core [B, FS] -> fhl [P, CH*KO, 2*B] bf16 matching the kernel's
    k order (k = ch*CHUNK_ROWS + p*KO + o at fhl[p, ch*KO + o]); the
    last axis holds fh[b] in [0, B) and fl[b] in [B, 2B)."""
    hi, lo = _split_hl(f_core)
    ft = np.concatenate([hi.T, lo.T], axis=1)          # [FS, 2B]
    ft = ft.reshape(CH, P, KO, 2 * B).transpose(1, 0, 2, 3)
    return np.ascontiguousarray(ft.reshape(P, CH * KO, 2 * B))


def _prep_w(w_core: np.ndarray) -> np.ndarray:
    """w_core [L, FS, D] -> [L, CH, P, KO, 2, D] bf16 (exact SBUF image)."""
    hi, lo = _split_hl(w_core)
    whl = np.stack([hi, lo], axis=2)                   # [L, FS, 2, D]
    whl = whl.reshape(L, CH, P, KO, 2, D)
    return np.ascontiguousarray(whl)


def kernel(f: np.ndarray, weight: np.ndarray, bias: np.ndarray) -> np.ndarray:
    f = np.asarray(f, dtype=np.float32)
    weight = np.asarray(weight, dtype=np.float32)
    bias = np.asarray(bias, dtype=np.float32)

    if "nc" not in _cache:
        _cache["nc"] = _build()
    nc = _cache["nc"]

    in_maps = []
    for c in range(NCORES):
        sl = slice(c * FS, (c + 1) * FS)
        in_maps.append(
            {
                "fhl": _prep_f(f[:, sl]),
                "w": _prep_w(weight[:, sl, :]),
            }
        )

    res = bass_utils.run_bass_kernel_spmd(nc, in_maps, core_ids=list(range(NCORES)))
    partial = np.stack([r["out"] for r in res.results])  # [NCORES, L, 2B, D]
    total = partial.sum(axis=0)                          # [L, 2B, D]
    total = total[:, :B, :] + total[:, B:, :]            # hi-half + lo-half
    x = total.transpose(1, 0, 2) + bias[None, :, :]      # [B, L, D]
    return x.astype(np.float32)
